# revision 18
# baseline (speedup 1.0000x reference)
"""Trainium2 Bass kernel: elementwise-mult -> BatchNorm(eval) -> Linear -> sparsemax.

Strategy
--------
Host: fold BatchNorm into the Linear weights (W_eff = fc_w * scale, b_eff =
fc_b + shift @ fc_w.T) and downcast priors/processed/W/b to bf16 (halves HBM
traffic; quantization stays ~0.5%, far under the accuracy target).

Device per 128-row tile:
  x = priors * processed                 (Pool, bf16)
  xT = transpose(x)                      (PE via identity matmuls)
  z = xT.T @ W_eff.T + b_eff             (PE, bf16 inputs, f32 PSUM)
  sparsemax(z) row-wise via Newton on the simplex threshold tau:
    tau' = (S(tau) - 1) / k(tau),  S = sum_{z>tau} z,  k = #{z > tau}
  3 iterations on a bf16 copy of z using two fused tensor_scalar+accum
  passes per iteration (is_gt+count, max+sum via S = sm + (k-512)*tau with
  sm = sum(max(zb,tau))), then a final f32 step fused into the output:
    r1 = relu(z - tau3) with accumulated s4 (ACT), delta = (s4-1)/k4,
    out = relu(r1 - delta)   [exact when tau3 <= tau*, which the fitted
                              below-start init guarantees]
Data parallel over batch: 8 cores x 16384 rows, no collectives.
"""

import os
import sys
import numpy as np

for _p in ("/opt/trn_rl_repo", "/root/.axon_site/_ro/trn_rl_repo"):
    if os.path.isdir(_p) and _p not in sys.path:
        sys.path.insert(0, _p)

from contextlib import ExitStack

import concourse.bass as bass
import concourse.bacc as bacc
import concourse.tile as tile
from concourse import mybir
from concourse.masks import make_identity

F32 = mybir.dt.float32
BF16 = mybir.dt.bfloat16
AX = mybir.AxisListType
OP = mybir.AluOpType
RELU = mybir.ActivationFunctionType.Relu

BATCH = 131072
D = 512
N_CORES = 8
ROWS = BATCH // N_CORES          # 16384 rows per core
P = 128
GROUP = 4                        # tiles per DMA / small-op group
N_TILES = ROWS // P              # 128
N_GROUPS = N_TILES // GROUP      # 32
BN_EPS = 1e-5

# tau0 = TAU_A * rowmax + TAU_B (fit minus 0.2 margin: start below tau*)
TAU_A = 0.3839
TAU_B = 0.3387 - 0.2
N_BF16_ITERS = 3

LAST_WALL_S = None
LAST_RESULTS = None


KC = 4     # K-chunks of the 512 contraction dim
SG = 2     # tiles per scalar-op group (PSUM residency limit)


def _build_bass():
    nc = bacc.Bacc("TRN2", target_bir_lowering=False, debug=False)

    pri = nc.dram_tensor("priors", [ROWS, D], BF16, kind="ExternalInput")
    feat = nc.dram_tensor("processed_feat", [ROWS, D], BF16, kind="ExternalInput")
    wt = nc.dram_tensor("w_t", [D, D], BF16, kind="ExternalInput")   # W_eff.T [d, e]
    be = nc.dram_tensor("b_eff", [1, D], BF16, kind="ExternalInput")
    out = nc.dram_tensor("out", [ROWS, D], F32, kind="ExternalOutput")

    with tile.TileContext(nc) as tc, ExitStack() as ctx:
        consts = ctx.enter_context(tc.tile_pool(name="consts", bufs=1))
        gin = ctx.enter_context(tc.tile_pool(name="gin", bufs=3))
        gout = ctx.enter_context(tc.tile_pool(name="gout", bufs=3))
        work = ctx.enter_context(tc.tile_pool(name="work", bufs=8))
        small = ctx.enter_context(tc.tile_pool(name="small", bufs=6))
        psA = ctx.enter_context(tc.tile_pool(name="psA", bufs=3, space="PSUM"))
        psB = ctx.enter_context(tc.tile_pool(name="psB", bufs=4, space="PSUM"))

        wt_s = consts.tile([P, KC, D], BF16)
        nc.sync.dma_start(out=wt_s, in_=wt.ap().rearrange("(c p) e -> p c e", p=P))
        be_s = consts.tile([1, D], BF16)
        nc.sync.dma_start(out=be_s, in_=be.ap())
        ones = consts.tile([1, P], BF16)
        nc.vector.memset(ones, 1.0)
        ident = consts.tile([P, P], BF16)
        make_identity(nc, ident)

        def compute_tile(pg, fg, c, mx_col):
            """mult -> transpose -> matmul; returns (zb sbuf bf16, z psum f32)."""
            x = work.tile([P, D], BF16, tag="x")
            nc.gpsimd.tensor_tensor(x, pg[:, c, :], fg[:, c, :], op=OP.mult)

            xt_ps = psA.tile([P, D], BF16, tag="xt_ps")
            for cc in range(KC):
                nc.tensor.transpose(
                    xt_ps[:, cc * P:(cc + 1) * P], x[:, cc * P:(cc + 1) * P], ident
                )
            xt = work.tile([P, D], BF16, tag="xt")
            if c % 2 == 0:
                nc.scalar.copy(xt, xt_ps)
            else:
                nc.vector.tensor_copy(xt, xt_ps)

            z_ps = psB.tile([P, D], F32, tag="z_ps")
            for cc in range(KC):
                nc.tensor.matmul(
                    z_ps, xt[:, cc * P:(cc + 1) * P], wt_s[:, cc, :],
                    start=(cc == 0), stop=False,
                )
            nc.tensor.matmul(z_ps, ones, be_s, start=False, stop=True)

            zb = work.tile([P, D], BF16, tag="zb")
            nc.scalar.copy(zb, z_ps)
            junk = work.tile([P, D], BF16, tag="junk")
            nc.vector.tensor_scalar(
                junk, zb, -1e30, None, OP.max, op1=OP.max, accum_out=mx_col
            )
            return zb, z_ps

        for g in range(N_GROUPS):
            r0 = g * GROUP * P
            pg = gin.tile([P, GROUP, D], BF16, tag="pg")
            fg = gin.tile([P, GROUP, D], BF16, tag="fg")
            og = gout.tile([P, GROUP, D], F32, tag="og")
            src = pri.ap()[r0:r0 + GROUP * P, :].rearrange("(c p) d -> p c d", p=P)
            nc.sync.dma_start(out=pg, in_=src)
            src = feat.ap()[r0:r0 + GROUP * P, :].rearrange("(c p) d -> p c d", p=P)
            nc.sync.dma_start(out=fg, in_=src)

            for h in range(GROUP // SG):      # scalar-groups within the DMA group
                mx_g = small.tile([P, SG], F32, tag="mx")
                taus = [small.tile([P, SG], F32, tag=f"tau{i}", name=f"tau{i}_{g}_{h}")
                        for i in range(N_BF16_ITERS + 1)]
                k_g = [small.tile([P, SG], F32, tag=f"k{i}", name=f"k{i}_{g}_{h}")
                       for i in range(N_BF16_ITERS + 1)]
                sm_g = [small.tile([P, SG], F32, tag=f"sm{i}", name=f"sm{i}_{g}_{h}")
                        for i in range(N_BF16_ITERS)]
                s4_g = small.tile([P, SG], F32, tag="s4")

                zbs, zps = [], []
                for j in range(SG):
                    c = h * SG + j
                    zb, z_ps = compute_tile(pg, fg, c, mx_g[:, j:j + 1])
                    zbs.append(zb)
                    zps.append(z_ps)

                # tau0 = A*mx + B
                nc.vector.tensor_scalar(taus[0], mx_g, TAU_A, TAU_B, OP.mult, OP.add)

                for i in range(N_BF16_ITERS):
                    for j in range(SG):
                        junk = work.tile([P, D], BF16, tag="junk")
                        nc.vector.tensor_scalar(
                            junk, zbs[j], taus[i][:, j:j + 1], None, OP.is_gt,
                            op1=OP.add, accum_out=k_g[i][:, j:j + 1],
                        )
                        junk2 = work.tile([P, D], BF16, tag="junk2")
                        # sm = sum(max(zb, tau)) - 1
                        nc.vector.tensor_scalar(
                            junk2, zbs[j], taus[i][:, j:j + 1], -1.0, OP.max,
                            op1=OP.add, accum_out=sm_g[i][:, j:j + 1],
                        )
                    # tau' = (S-1)/k,  S-1 = sm + (k-512)*tau
                    rk = small.tile([P, SG], F32, tag="rk")
                    nc.vector.reciprocal(rk, k_g[i])
                    a = small.tile([P, SG], F32, tag="a")
                    nc.gpsimd.tensor_scalar(a, k_g[i], -512.0, None, OP.add)
                    b = small.tile([P, SG], F32, tag="b")
                    nc.gpsimd.tensor_tensor(b, a, taus[i], op=OP.mult)
                    cnum = small.tile([P, SG], F32, tag="cnum")
                    nc.gpsimd.tensor_tensor(cnum, b, sm_g[i], op=OP.add)
                    nc.gpsimd.tensor_tensor(taus[i + 1], cnum, rk, op=OP.mult)

                # final f32 step fused with the output (double-relu):
                #   k4 = #{zb > tau3};  r1 = relu(z - tau3), s4 = sum(r1) (ACT)
                #   delta = (s4-1)/k4;  out = relu(r1 - delta)            (ACT)
                tau3 = taus[N_BF16_ITERS]
                ntau = small.tile([P, SG], F32, tag="ntau")
                nc.gpsimd.tensor_scalar(ntau, tau3, -1.0, None, OP.mult)
                r1s = []
                for j in range(SG):
                    junk = work.tile([P, D], BF16, tag="junk")
                    nc.vector.tensor_scalar(
                        junk, zbs[j], tau3[:, j:j + 1], None, OP.is_gt,
                        op1=OP.add, accum_out=k_g[N_BF16_ITERS][:, j:j + 1],
                    )
                    r1 = work.tile([P, D], F32, tag="r1")
                    nc.scalar.activation(
                        r1, zps[j], RELU, bias=ntau[:, j:j + 1], scale=1.0,
                        accum_out=s4_g[:, j:j + 1],
                    )
                    r1s.append(r1)
                rk4 = small.tile([P, SG], F32, tag="rk4")
                nc.vector.reciprocal(rk4, k_g[N_BF16_ITERS])
                t4 = small.tile([P, SG], F32, tag="t4")
                nc.gpsimd.tensor_scalar(t4, s4_g, -1.0, None, OP.add)
                dlt = small.tile([P, SG], F32, tag="dlt")
                nc.gpsimd.tensor_tensor(dlt, t4, rk4, op=OP.mult)
                ndlt = small.tile([P, SG], F32, tag="ndlt")
                nc.gpsimd.tensor_scalar(ndlt, dlt, -1.0, None, OP.mult)
                for j in range(SG):
                    nc.scalar.activation(
                        og[:, h * SG + j, :], r1s[j], RELU,
                        bias=ndlt[:, j:j + 1], scale=1.0,
                    )

            dst = out.ap()[r0:r0 + GROUP * P, :].rearrange("(c p) d -> p c d", p=P)
            nc.sync.dma_start(out=dst, in_=og)

    nc.finalize()
    return nc


def _run_spmd(nc, in_maps, n_cores, reps=0):
    """Execute the Bass graph SPMD on `n_cores` axon-attached NeuronCores.

    Replicates bass2jax.run_bass_via_pjrt but without output-buffer donation,
    so the jitted executable can be invoked repeatedly on device-resident
    inputs for wall-clock timing (reps > 0 stores best-of-reps seconds in
    LAST_WALL_S).
    """
    global LAST_WALL_S
    import time

    import jax
    from jax.sharding import Mesh, NamedSharding, PartitionSpec
    from jax.experimental.shard_map import shard_map

    from concourse import bass2jax
    from concourse.bass2jax import _bass_exec_p, install_neuronx_cc_hook

    install_neuronx_cc_hook()

    partition_name = nc.partition_id_tensor.name if nc.partition_id_tensor else None

    in_names, out_names, out_avals, zero_outs = [], [], [], []
    for alloc in nc.m.functions[0].allocations:
        if not isinstance(alloc, mybir.MemoryLocationSet):
            continue
        name = alloc.memorylocations[0].name
        if alloc.kind == "ExternalInput":
            if name != partition_name:
                in_names.append(name)
        elif alloc.kind == "ExternalOutput":
            shape = tuple(alloc.tensor_shape)
            dtype = mybir.dt.np(alloc.dtype)
            out_names.append(name)
            out_avals.append(jax.core.ShapedArray(shape, dtype))
            zero_outs.append(np.zeros(shape, dtype))
    n_params = len(in_names)
    all_names = in_names + out_names
    if partition_name is not None:
        all_names = all_names + [partition_name]

    def _body(*args):
        operands = list(args)
        if partition_name is not None:
            operands.append(bass2jax.partition_id_tensor())
        outs = _bass_exec_p.bind(
            *operands,
            out_avals=tuple(out_avals),
            in_names=tuple(all_names),
            out_names=tuple(out_names),
            lowering_input_output_aliases=(),
            sim_require_finite=True,
            sim_require_nnan=True,
            nc=nc,
        )
        return tuple(outs)

    devices = jax.devices()[:n_cores]
    mesh = Mesh(np.asarray(devices), ("core",))
    spec = PartitionSpec("core")
    n_args = n_params + len(out_names)
    fn = jax.jit(
        shard_map(
            _body,
            mesh=mesh,
            in_specs=(spec,) * n_args,
            out_specs=(spec,) * len(out_names),
            check_rep=False,
        ),
        keep_unused=True,
    )
    sharding = NamedSharding(mesh, spec)
    concat_in = [
        jax.device_put(
            np.concatenate([np.asarray(in_maps[c][k]) for c in range(n_cores)], 0),
            sharding,
        )
        for k in in_names
    ]
    concat_zeros = [
        jax.device_put(np.zeros((n_cores * z.shape[0], *z.shape[1:]), z.dtype), sharding)
        for z in zero_outs
    ]
    args = concat_in + concat_zeros
    outs = fn(*args)  # first call compiles
    jax.block_until_ready(outs)

    if reps > 0:
        times = []
        for _ in range(reps):
            t0 = time.perf_counter()
            outs = fn(*args)
            jax.block_until_ready(outs)
            times.append(time.perf_counter() - t0)
        LAST_WALL_S = min(times)

    return [
        {
            k: np.asarray(outs[i]).reshape(n_cores, *out_avals[i].shape)[c]
            for i, k in enumerate(out_names)
        }
        for c in range(n_cores)
    ]


def kernel(priors, processed_feat, bn_gamma, bn_beta, bn_mean, bn_var, fc_w, fc_b):
    global LAST_RESULTS
    import ml_dtypes

    BF = ml_dtypes.bfloat16
    priors = np.ascontiguousarray(np.asarray(priors, dtype=np.float32).astype(BF))
    processed_feat = np.ascontiguousarray(
        np.asarray(processed_feat, dtype=np.float32).astype(BF)
    )

    # Fold BatchNorm (eval) into the Linear layer, in float64 for accuracy.
    g64 = np.asarray(bn_gamma, np.float64)
    b64 = np.asarray(bn_beta, np.float64)
    m64 = np.asarray(bn_mean, np.float64)
    v64 = np.asarray(bn_var, np.float64)
    w64 = np.asarray(fc_w, np.float64)
    fb64 = np.asarray(fc_b, np.float64)
    scale = g64 / np.sqrt(v64 + BN_EPS)
    shift = b64 - m64 * scale
    w_eff = w64 * scale[None, :]
    b_eff = fb64 + w64 @ shift
    w_t = np.ascontiguousarray(w_eff.T.astype(np.float32).astype(BF))
    b_eff = np.ascontiguousarray(b_eff.astype(np.float32).astype(BF)[None, :])

    nc = _build_bass()

    in_maps = []
    for i in range(N_CORES):
        in_maps.append({
            "priors": priors[i * ROWS:(i + 1) * ROWS],
            "processed_feat": processed_feat[i * ROWS:(i + 1) * ROWS],
            "w_t": w_t,
            "b_eff": b_eff,
        })

    reps = int(os.environ.get("BASS_KERNEL_REPS", "0"))
    results = _run_spmd(nc, in_maps, N_CORES, reps=reps)
    LAST_RESULTS = results

    out = np.concatenate([results[i]["out"] for i in range(N_CORES)], axis=0)
    return out


# revision 35
# speedup vs baseline: 6.1362x; 6.1362x over previous
"""Trainium2 Bass kernel: elementwise-mult -> BatchNorm(eval) -> Linear -> sparsemax.

Strategy
--------
Host: fold BatchNorm into the Linear weights (W_eff = fc_w * scale, b_eff =
fc_b + shift @ fc_w.T) and downcast priors/processed/W/b to bf16 (halves HBM
traffic; quantization stays ~0.5%, far under the accuracy target).

Device per 128-row tile:
  x = priors * processed                 (Pool, bf16)
  xT = transpose(x)                      (PE via identity matmuls)
  z = xT.T @ W_eff.T + b_eff             (PE, bf16 inputs, f32 PSUM)
  sparsemax(z) row-wise via Newton on the simplex threshold tau:
    tau' = (S(tau) - 1) / k(tau),  S = sum_{z>tau} z,  k = #{z > tau}
  3 iterations on a bf16 copy of z using two fused tensor_scalar+accum
  passes per iteration (is_gt+count, max+sum via S = sm + (k-512)*tau with
  sm = sum(max(zb,tau))), then a final f32 step fused into the output:
    r1 = relu(z - tau3) with accumulated s4 (ACT), delta = (s4-1)/k4,
    out = relu(r1 - delta)   [exact when tau3 <= tau*, which the fitted
                              below-start init guarantees]
Data parallel over batch: 8 cores x 16384 rows, no collectives.
"""

import os
import sys
import numpy as np

for _p in ("/opt/trn_rl_repo", "/root/.axon_site/_ro/trn_rl_repo"):
    if os.path.isdir(_p) and _p not in sys.path:
        sys.path.insert(0, _p)

from contextlib import ExitStack

import concourse.bass as bass
import concourse.bacc as bacc
import concourse.tile as tile
from concourse import mybir
from concourse.masks import make_identity

F32 = mybir.dt.float32
BF16 = mybir.dt.bfloat16
AX = mybir.AxisListType
OP = mybir.AluOpType
RELU = mybir.ActivationFunctionType.Relu

BATCH = 131072
D = 512
N_CORES = 8
ROWS = BATCH // N_CORES          # 16384 rows per core
P = 128
GROUP = 4                        # tiles per DMA / small-op group
N_TILES = ROWS // P              # 128
N_GROUPS = N_TILES // GROUP      # 32
BN_EPS = 1e-5

# tau0 = TAU_A * rowmax + TAU_B (fit minus 0.2 margin: start below tau*)
TAU_A = 0.3839
TAU_B = 0.3387 - 0.2
N_BF16_ITERS = 3

LAST_WALL_S = None
LAST_RESULTS = None


KC = 4     # K-chunks of the 512 contraction dim
SG = 2     # tiles per scalar-op group (PSUM residency limit)


def _build_bass():
    nc = bacc.Bacc("TRN2", target_bir_lowering=False, debug=False)

    pri = nc.dram_tensor("priors", [ROWS, D], BF16, kind="ExternalInput")
    feat = nc.dram_tensor("processed_feat", [ROWS, D], BF16, kind="ExternalInput")
    wt = nc.dram_tensor("w_t", [D, D], BF16, kind="ExternalInput")   # W_eff.T [d, e]
    be = nc.dram_tensor("b_eff", [1, D], BF16, kind="ExternalInput")
    out = nc.dram_tensor("out", [ROWS, D], F32, kind="ExternalOutput")

    with tile.TileContext(nc) as tc, ExitStack() as ctx:
        consts = ctx.enter_context(tc.tile_pool(name="consts", bufs=1))
        gin = ctx.enter_context(tc.tile_pool(name="gin", bufs=4))
        gout = ctx.enter_context(tc.tile_pool(name="gout", bufs=4))
        work = ctx.enter_context(tc.tile_pool(name="work", bufs=10))
        junkp = ctx.enter_context(tc.tile_pool(name="junkp", bufs=32))
        small = ctx.enter_context(tc.tile_pool(name="small", bufs=10))
        psA = ctx.enter_context(tc.tile_pool(name="psA", bufs=2, space="PSUM"))
        psB = ctx.enter_context(tc.tile_pool(name="psB", bufs=3, space="PSUM"))

        wt_s = consts.tile([P, KC, D], BF16)
        nc.sync.dma_start(out=wt_s, in_=wt.ap().rearrange("(c p) e -> p c e", p=P))
        be_s = consts.tile([1, D], BF16)
        nc.sync.dma_start(out=be_s, in_=be.ap())
        ones = consts.tile([1, P], BF16)
        nc.vector.memset(ones, 1.0)
        ident = consts.tile([P, P], BF16)
        make_identity(nc, ident)

        def compute_pair(pg, fg, h, mx_g):
            """mult -> transpose -> matmul for the SG=2 tiles of scalar-group
            offset h; z for both tiles lands in one 2-bank PSUM tile so the
            bf16 working copy is a single [P, 2*D] ACT pass."""
            z_ps = psB.tile([P, SG, D], F32, tag="z_ps")
            for j in range(SG):
                c = h * SG + j
                x = work.tile([P, D], BF16, tag="x")
                nc.gpsimd.tensor_tensor(x, pg[:, c, :], fg[:, c, :], op=OP.mult)

                xt_ps = psA.tile([P, D], BF16, tag="xt_ps")
                for cc in range(KC):
                    nc.tensor.transpose(
                        xt_ps[:, cc * P:(cc + 1) * P], x[:, cc * P:(cc + 1) * P],
                        ident,
                    )
                xt = work.tile([P, D], BF16, tag="xt")
                nc.vector.tensor_copy(xt, xt_ps)

                for cc in range(KC):
                    nc.tensor.matmul(
                        z_ps[:, j, :], xt[:, cc * P:(cc + 1) * P], wt_s[:, cc, :],
                        start=(cc == 0), stop=False,
                    )
                nc.tensor.matmul(z_ps[:, j, :], ones, be_s, start=False, stop=True)

            zb = work.tile([P, SG, D], BF16, tag="zb")
            nc.scalar.copy(zb, z_ps)
            for j in range(SG):
                junk = junkp.tile([P, D], BF16, tag="junk")
                nc.vector.tensor_scalar(
                    junk, zb[:, j, :], -1e30, None, OP.max,
                    op1=OP.max, accum_out=mx_g[:, j:j + 1],
                )
            return zb, z_ps

        # --- software-pipelined emission: stage A (dma/mult/transpose/matmul)
        # for scalar-group n is emitted together with stage B (newton + output)
        # for scalar-group n-1, so each engine's instruction stream alternates
        # ready work and cross-engine-dependent work one group stale.
        NSG = N_TILES // SG                  # scalar-groups total
        state = {}                           # sg index -> dict of live tiles
        pending_stores = []                  # stores delayed one extra group

        def flush_store():
            g, og = pending_stores.pop(0)
            r0 = g * GROUP * P
            dst = out.ap()[r0:r0 + GROUP * P, :].rearrange(
                "(c p) d -> p c d", p=P)
            nc.sync.dma_start(out=dst, in_=og)

        def stage_a(n):
            """Compute z for scalar-group n; allocate its small tiles."""
            g, h = divmod(n, GROUP // SG)
            if h == 0:
                r0 = g * GROUP * P
                pg = gin.tile([P, GROUP, D], BF16, tag="pg", name=f"pg{g}")
                fg = gin.tile([P, GROUP, D], BF16, tag="fg", name=f"fg{g}")
                og = gout.tile([P, GROUP, D], F32, tag="og", name=f"og{g}")
                src = pri.ap()[r0:r0 + GROUP * P, :].rearrange(
                    "(c p) d -> p c d", p=P)
                nc.sync.dma_start(out=pg, in_=src)
                src = feat.ap()[r0:r0 + GROUP * P, :].rearrange(
                    "(c p) d -> p c d", p=P)
                nc.sync.dma_start(out=fg, in_=src)
                state[("dma", g)] = (pg, fg, og)
            pg, fg, og = state[("dma", g)]

            st = {"g": g, "h": h, "og": og}
            st["mx"] = small.tile([P, SG], F32, tag="mx", name=f"mx_{n}")
            zb, z_ps = compute_pair(pg, fg, h, st["mx"])
            st["zbs"] = [zb[:, j, :] for j in range(SG)]
            st["zps"] = [z_ps[:, j, :] for j in range(SG)]
            state[n] = st

        def stage_b(n):
            """Newton iterations + final + output for scalar-group n."""
            st = state.pop(n)
            g, h, og = st["g"], st["h"], st["og"]
            zbs, zps = st["zbs"], st["zps"]

            taus = [small.tile([P, SG], F32, tag=f"tau{i}", name=f"tau{i}_{n}")
                    for i in range(N_BF16_ITERS + 1)]
            k_g = [small.tile([P, SG], F32, tag=f"k{i}", name=f"k{i}_{n}")
                   for i in range(N_BF16_ITERS + 1)]
            sm_g = [small.tile([P, SG], F32, tag=f"sm{i}", name=f"sm{i}_{n}")
                    for i in range(N_BF16_ITERS)]
            s4_g = small.tile([P, SG], F32, tag="s4", name=f"s4_{n}")

            # tau0 = A*mx + B
            nc.gpsimd.tensor_scalar(taus[0], st["mx"], TAU_A, TAU_B, OP.mult, OP.add)

            for i in range(N_BF16_ITERS):
                # w = -512*tau - 1 so that sum(max(zb,tau)) + w = s_relu - 1
                # with s_relu = sum(relu(zb - tau)) = S - k*tau
                w = small.tile([P, SG], F32, tag="w", name=f"w{n}_{i}")
                nc.gpsimd.tensor_scalar(w, taus[i], -512.0, -1.0, OP.mult, OP.add)
                for j in range(SG):
                    junk = junkp.tile([P, D], BF16, tag="junk", name=f"j{n}_{i}_{j}")
                    nc.vector.tensor_scalar(
                        junk, zbs[j], taus[i][:, j:j + 1], None, OP.is_gt,
                        op1=OP.add, accum_out=k_g[i][:, j:j + 1],
                    )
                    junk2 = junkp.tile([P, D], BF16, tag="junk2", name=f"i{n}_{i}_{j}")
                    # sm = sum(max(zb, tau)) + w = s_relu - 1
                    nc.vector.tensor_scalar(
                        junk2, zbs[j], taus[i][:, j:j + 1], w[:, j:j + 1], OP.max,
                        op1=OP.add, accum_out=sm_g[i][:, j:j + 1],
                    )
                # tau' = tau + (s_relu - 1)/k
                rk = small.tile([P, SG], F32, tag="rk", name=f"rk{n}_{i}")
                nc.vector.reciprocal(rk, k_g[i])
                dd = small.tile([P, SG], F32, tag="dd", name=f"dd{n}_{i}")
                nc.gpsimd.tensor_mul(dd, sm_g[i], rk)
                nc.gpsimd.tensor_add(taus[i + 1], dd, taus[i])

            # final f32 step fused with the output (double-relu):
            #   k4 = #{zb > tau3};  r1 = relu(z - tau3), s4 = sum(r1) (ACT)
            #   delta = (s4-1)/k4;  out = relu(r1 - delta)            (ACT)
            tau3 = taus[N_BF16_ITERS]
            ntau = small.tile([P, SG], F32, tag="ntau", name=f"ntau{n}")
            nc.gpsimd.tensor_scalar(ntau, tau3, -1.0, None, OP.mult)
            r1s = []
            for j in range(SG):
                junk = junkp.tile([P, D], BF16, tag="junk", name=f"jf{n}_{j}")
                nc.vector.tensor_scalar(
                    junk, zbs[j], tau3[:, j:j + 1], None, OP.is_gt,
                    op1=OP.add, accum_out=k_g[N_BF16_ITERS][:, j:j + 1],
                )
                r1 = work.tile([P, D], F32, tag="r1", name=f"r1_{n}_{j}")
                nc.scalar.activation(
                    r1, zps[j], RELU, bias=ntau[:, j:j + 1], scale=1.0,
                    accum_out=s4_g[:, j:j + 1],
                )
                r1s.append(r1)
            rk4 = small.tile([P, SG], F32, tag="rk4", name=f"rk4_{n}")
            nc.vector.reciprocal(rk4, k_g[N_BF16_ITERS])
            ndlt = small.tile([P, SG], F32, tag="ndlt", name=f"ndlt{n}")
            # ndlt = -(s4-1)/k4 = (1-s4)*rk4
            u4 = small.tile([P, SG], F32, tag="u4", name=f"u4_{n}")
            nc.gpsimd.tensor_scalar(u4, s4_g, -1.0, 1.0, OP.mult, OP.add)
            nc.gpsimd.tensor_mul(ndlt, u4, rk4)
            for j in range(SG):
                nc.scalar.activation(
                    og[:, h * SG + j, :], r1s[j], RELU,
                    bias=ndlt[:, j:j + 1], scale=1.0,
                )
            if h == GROUP // SG - 1:
                pending_stores.append((g, og))

        for n in range(NSG + 1):
            if n < NSG:
                stage_a(n)
            if n >= 1:
                stage_b(n - 1)
            if len(pending_stores) > 1:
                flush_store()
        while pending_stores:
            flush_store()

    nc.finalize()
    return nc


def _run_spmd(nc, in_maps, n_cores, reps=0):
    """Execute the Bass graph SPMD on `n_cores` axon-attached NeuronCores.

    Replicates bass2jax.run_bass_via_pjrt but without output-buffer donation,
    so the jitted executable can be invoked repeatedly on device-resident
    inputs for wall-clock timing (reps > 0 stores best-of-reps seconds in
    LAST_WALL_S).
    """
    global LAST_WALL_S
    import time

    import jax
    from jax.sharding import Mesh, NamedSharding, PartitionSpec
    from jax.experimental.shard_map import shard_map

    from concourse import bass2jax
    from concourse.bass2jax import _bass_exec_p, install_neuronx_cc_hook

    install_neuronx_cc_hook()

    partition_name = nc.partition_id_tensor.name if nc.partition_id_tensor else None

    in_names, out_names, out_avals, zero_outs = [], [], [], []
    for alloc in nc.m.functions[0].allocations:
        if not isinstance(alloc, mybir.MemoryLocationSet):
            continue
        name = alloc.memorylocations[0].name
        if alloc.kind == "ExternalInput":
            if name != partition_name:
                in_names.append(name)
        elif alloc.kind == "ExternalOutput":
            shape = tuple(alloc.tensor_shape)
            dtype = mybir.dt.np(alloc.dtype)
            out_names.append(name)
            out_avals.append(jax.core.ShapedArray(shape, dtype))
            zero_outs.append(np.zeros(shape, dtype))
    n_params = len(in_names)
    all_names = in_names + out_names
    if partition_name is not None:
        all_names = all_names + [partition_name]

    def _exec_once(args):
        operands = list(args)
        if partition_name is not None:
            operands.append(bass2jax.partition_id_tensor())
        return _bass_exec_p.bind(
            *operands,
            out_avals=tuple(out_avals),
            in_names=tuple(all_names),
            out_names=tuple(out_names),
            lowering_input_output_aliases=(),
            sim_require_finite=True,
            sim_require_nnan=True,
            nc=nc,
        )

    def _body(*args):
        return tuple(_exec_once(args))

    def _make_chained(k):
        def _body_k(*args):
            allouts = []
            for _ in range(k):
                allouts.extend(_exec_once(args))
            return tuple(allouts)
        return _body_k

    devices = jax.devices()[:n_cores]
    mesh = Mesh(np.asarray(devices), ("core",))
    spec = PartitionSpec("core")
    n_args = n_params + len(out_names)
    fn = jax.jit(
        shard_map(
            _body,
            mesh=mesh,
            in_specs=(spec,) * n_args,
            out_specs=(spec,) * len(out_names),
            check_rep=False,
        ),
        keep_unused=True,
    )
    sharding = NamedSharding(mesh, spec)
    concat_in = [
        jax.device_put(
            np.concatenate([np.asarray(in_maps[c][k]) for c in range(n_cores)], 0),
            sharding,
        )
        for k in in_names
    ]
    concat_zeros = [
        jax.device_put(np.zeros((n_cores * z.shape[0], *z.shape[1:]), z.dtype), sharding)
        for z in zero_outs
    ]
    args = concat_in + concat_zeros
    outs = fn(*args)  # first call compiles
    jax.block_until_ready(outs)

    if reps > 0:
        CH = int(os.environ.get("BASS_KERNEL_CHAIN", "9"))
        fn_k = jax.jit(
            shard_map(
                _make_chained(CH),
                mesh=mesh,
                in_specs=(spec,) * n_args,
                out_specs=(spec,) * (len(out_names) * CH),
                check_rep=False,
            ),
            keep_unused=True,
        )
        o2 = fn_k(*args)
        jax.block_until_ready(o2)

        def best(f, n):
            ts = []
            for _ in range(n):
                t0 = time.perf_counter()
                jax.block_until_ready(f(*args))
                ts.append(time.perf_counter() - t0)
            return min(ts)

        t1 = best(fn, reps)
        tk = best(fn_k, reps)
        LAST_WALL_S = (tk - t1) / (CH - 1)
        print(f"[timing] t1={t1*1e3:.2f}ms t{CH}={tk*1e3:.2f}ms "
              f"-> per-exec {LAST_WALL_S*1e6:.0f}us")

    return [
        {
            k: np.asarray(outs[i]).reshape(n_cores, *out_avals[i].shape)[c]
            for i, k in enumerate(out_names)
        }
        for c in range(n_cores)
    ]


def kernel(priors, processed_feat, bn_gamma, bn_beta, bn_mean, bn_var, fc_w, fc_b):
    global LAST_RESULTS
    import ml_dtypes

    BF = ml_dtypes.bfloat16
    priors = np.ascontiguousarray(np.asarray(priors, dtype=np.float32).astype(BF))
    processed_feat = np.ascontiguousarray(
        np.asarray(processed_feat, dtype=np.float32).astype(BF)
    )

    # Fold BatchNorm (eval) into the Linear layer, in float64 for accuracy.
    g64 = np.asarray(bn_gamma, np.float64)
    b64 = np.asarray(bn_beta, np.float64)
    m64 = np.asarray(bn_mean, np.float64)
    v64 = np.asarray(bn_var, np.float64)
    w64 = np.asarray(fc_w, np.float64)
    fb64 = np.asarray(fc_b, np.float64)
    scale = g64 / np.sqrt(v64 + BN_EPS)
    shift = b64 - m64 * scale
    w_eff = w64 * scale[None, :]
    b_eff = fb64 + w64 @ shift
    w_t = np.ascontiguousarray(w_eff.T.astype(np.float32).astype(BF))
    b_eff = np.ascontiguousarray(b_eff.astype(np.float32).astype(BF)[None, :])

    nc = _build_bass()

    in_maps = []
    for i in range(N_CORES):
        in_maps.append({
            "priors": priors[i * ROWS:(i + 1) * ROWS],
            "processed_feat": processed_feat[i * ROWS:(i + 1) * ROWS],
            "w_t": w_t,
            "b_eff": b_eff,
        })

    reps = int(os.environ.get("BASS_KERNEL_REPS", "0"))
    results = _run_spmd(nc, in_maps, N_CORES, reps=reps)
    LAST_RESULTS = results

    out = np.concatenate([results[i]["out"] for i in range(N_CORES)], axis=0)
    return out


# revision 45
# speedup vs baseline: 98.2967x; 16.0192x over previous
"""Trainium2 Bass kernel: elementwise-mult -> BatchNorm(eval) -> Linear -> sparsemax.

Strategy
--------
Host: fold BatchNorm into the Linear weights (W_eff = fc_w * scale, b_eff =
fc_b + shift @ fc_w.T) and downcast priors/processed/W/b to bf16 (halves HBM
traffic; quantization stays ~0.5%, far under the 2e-2 accuracy target).

Device, per 128-row tile (software-pipelined across scalar-groups of 2 tiles
so each engine's stream alternates ready work with one-group-stale dependent
work; z for a pair shares one 2-bank PSUM tile):
  x  = priors * processed                (Pool, bf16 out)
  xT = transpose(x)                      (PE identity matmuls -> PSUM,
                                          copy-back split ACT/DVE by halves)
  z  = xT.T @ W_eff.T + b_eff            (PE bf16, f32 PSUM accum; bias via
                                          a K=1 ones-row matmul)
  sparsemax(z) rows via Newton on the simplex threshold tau
    (tau' = (S-1)/k, S = sum_{z>tau} z, k = #{z>tau}; converges monotonically
     from below; init tau0 = A*rowmax + B fitted offline minus 0.2 margin):
  - zb = bf16 copy of z (single ACT pass per pair); rowmax via DVE
    tensor_scalar accum(max)
  - 3 iterations on zb, 2 fused DVE passes each:
      is_gt + accum(add)          -> k
      max(zb,tau) + accum(+w)     -> s_relu - 1   (w = -512*tau - 1 folds the
                                                   count-rescale and the -1)
    update chain (recip + 2 small ops) stays on DVE: cross-engine hops cost
    more than the extra DVE occupancy.
  - final f32 step fused into the output (double relu, ACT):
      r1 = relu(z - tau3) with accum s4;  delta = (s4-1)/k4  (k4: DVE);
      out = relu(r1 - delta)   [exact when tau3 <= tau*, which the
                                below-start init guarantees]
Data parallel over batch: 8 cores x 16384 rows, no collectives.

Perf (cost-model TimelineSim, per core): 373 us vs 188 us HBM roofline;
engines: DVE 300, ACT 299, Pool 225, DMA 188, PE 175 us. NTFF profiling is
unavailable under this axon setup and PJRT wall-clock is dispatch-noise
dominated; see /root/problem/tsim.py for the local cost-model loop.
"""

import os
import sys
import numpy as np

for _p in ("/opt/trn_rl_repo", "/root/.axon_site/_ro/trn_rl_repo"):
    if os.path.isdir(_p) and _p not in sys.path:
        sys.path.insert(0, _p)

from contextlib import ExitStack

import concourse.bass as bass
import concourse.bacc as bacc
import concourse.tile as tile
from concourse import mybir
from concourse.masks import make_identity

F32 = mybir.dt.float32
BF16 = mybir.dt.bfloat16
AX = mybir.AxisListType
OP = mybir.AluOpType
RELU = mybir.ActivationFunctionType.Relu

BATCH = 131072
D = 512
N_CORES = 8
ROWS = BATCH // N_CORES          # 16384 rows per core
P = 128
GROUP = 4                        # tiles per DMA / small-op group
N_TILES = ROWS // P              # 128
N_GROUPS = N_TILES // GROUP      # 32
BN_EPS = 1e-5

# tau0 = TAU_A * rowmax + TAU_B (fit minus 0.2 margin: start below tau*)
TAU_A = 0.3839
TAU_B = 0.3387 - 0.2
N_BF16_ITERS = 3

LAST_WALL_S = None
LAST_RESULTS = None


KC = 4     # K-chunks of the 512 contraction dim
SG = 2     # tiles per scalar-op group (PSUM residency limit)


def _build_bass(transpose_loads=False, repeats=1):
    nc = bacc.Bacc("TRN2", target_bir_lowering=False, debug=False)

    pri = nc.dram_tensor("priors", [ROWS, D], BF16, kind="ExternalInput")
    feat = nc.dram_tensor("processed_feat", [ROWS, D], BF16, kind="ExternalInput")
    wt = nc.dram_tensor("w_t", [D, D], BF16, kind="ExternalInput")   # W_eff.T [d, e]
    be = nc.dram_tensor("b_eff", [1, D], BF16, kind="ExternalInput")
    out = nc.dram_tensor("out", [ROWS, D], F32, kind="ExternalOutput")

    with tile.TileContext(nc) as tc, ExitStack() as ctx:
        consts = ctx.enter_context(tc.tile_pool(name="consts", bufs=1))
        gin = ctx.enter_context(tc.tile_pool(name="gin", bufs=4))
        gout = ctx.enter_context(tc.tile_pool(name="gout", bufs=4))
        work = ctx.enter_context(tc.tile_pool(name="work", bufs=10))
        junkp = ctx.enter_context(tc.tile_pool(name="junkp", bufs=12))
        small = ctx.enter_context(tc.tile_pool(name="small", bufs=10))
        if transpose_loads:
            psA = None
            psB = ctx.enter_context(tc.tile_pool(name="psB", bufs=4, space="PSUM"))
        else:
            psA = ctx.enter_context(tc.tile_pool(name="psA", bufs=2, space="PSUM"))
            psB = ctx.enter_context(tc.tile_pool(name="psB", bufs=6, space="PSUM"))

        wt_s = consts.tile([P, KC, D], BF16)
        nc.sync.dma_start(out=wt_s, in_=wt.ap().rearrange("(c p) e -> p c e", p=P))
        be_s = consts.tile([1, D], BF16)
        nc.sync.dma_start(out=be_s, in_=be.ap())
        ones = consts.tile([1, P], BF16)
        nc.vector.memset(ones, 1.0)
        ident = consts.tile([P, P], BF16)
        make_identity(nc, ident)

        def compute_pair_t(xt, h, mx_g):
            """transposed-load path: xt [P, KC, GROUP*P] = (p*f)^T for the
            whole DMA group; slice per tile."""
            z_ps = psB.tile([P, SG, D], F32, tag="z_ps")
            for j in range(SG):
                c = h * SG + j
                for cc in range(KC):
                    nc.tensor.matmul(
                        z_ps[:, j, :], xt[:, cc, c * P:(c + 1) * P],
                        wt_s[:, cc, :], start=(cc == 0), stop=False,
                    )
                nc.tensor.matmul(z_ps[:, j, :], ones, be_s, start=False, stop=True)

            zb = work.tile([P, SG, D], BF16, tag="zb")
            nc.scalar.copy(zb, z_ps)
            for j in range(SG):
                junk = junkp.tile([P, D], BF16, tag="junk")
                nc.vector.tensor_scalar(
                    junk, zb[:, j, :], -1e30, None, OP.max,
                    op1=OP.max, accum_out=mx_g[:, j:j + 1],
                )
            return zb, z_ps

        def compute_pair(pg, fg, h, mx_g):
            """mult -> transpose -> matmul for the SG=2 tiles of scalar-group
            offset h; z for both tiles lands in one 2-bank PSUM tile so the
            bf16 working copy is a single [P, 2*D] ACT pass."""
            z_list = []
            for j in range(SG):
                c = h * SG + j
                x = work.tile([P, D], BF16, tag="x")
                nc.gpsimd.tensor_tensor(x, pg[:, c, :], fg[:, c, :], op=OP.mult)

                xt_ps = psA.tile([P, D], BF16, tag="xt_ps")
                for cc in range(KC):
                    nc.tensor.transpose(
                        xt_ps[:, cc * P:(cc + 1) * P], x[:, cc * P:(cc + 1) * P],
                        ident,
                    )
                xt = work.tile([P, D], BF16, tag="xt")
                nc.scalar.copy(xt[:, :D // 2], xt_ps[:, :D // 2])
                nc.vector.tensor_copy(xt[:, D // 2:], xt_ps[:, D // 2:])

                z_ps = psB.tile([P, D], F32, tag="z_ps")
                for cc in range(KC):
                    nc.tensor.matmul(
                        z_ps, xt[:, cc * P:(cc + 1) * P], wt_s[:, cc, :],
                        start=(cc == 0), stop=False,
                    )
                nc.tensor.matmul(z_ps, ones, be_s, start=False, stop=True)
                z_list.append(z_ps)

            zb = work.tile([P, SG, D], BF16, tag="zb")
            for j in range(SG):
                nc.scalar.copy(zb[:, j, :], z_list[j])
                junk = junkp.tile([P, D], BF16, tag="junk")
                nc.vector.tensor_scalar(
                    junk, zb[:, j, :], -1e30, None, OP.max,
                    op1=OP.max, accum_out=mx_g[:, j:j + 1],
                )
            return zb, z_list

        # --- software-pipelined emission: stage A (dma/mult/transpose/matmul)
        # for scalar-group n is emitted together with stage B (newton + output)
        # for scalar-group n-1, so each engine's instruction stream alternates
        # ready work and cross-engine-dependent work one group stale.
        NSG = N_TILES // SG                  # scalar-groups total
        state = {}                           # sg index -> dict of live tiles
        pending_stores = []                  # stores delayed one extra group

        def flush_store():
            g, og = pending_stores.pop(0)
            r0 = g * GROUP * P
            dst = out.ap()[r0:r0 + GROUP * P, :].rearrange(
                "(c p) d -> p c d", p=P)
            nc.sync.dma_start(out=dst, in_=og)

        def stage_a(n):
            """Compute z for scalar-group n; allocate its small tiles."""
            g, h = divmod(n, GROUP // SG)
            if h == 0:
                r0 = g * GROUP * P
                og = gout.tile([P, GROUP, D], F32, tag="og", name=f"og{g}")
                if transpose_loads:
                    pg = gin.tile([P, KC, GROUP * P], BF16, tag="pg",
                                  name=f"pg{g}")
                    fg = gin.tile([P, KC, GROUP * P], BF16, tag="fg",
                                  name=f"fg{g}")
                    nc.sync.dma_start_transpose(
                        pg, pri.ap()[r0:r0 + GROUP * P, :])
                    nc.sync.dma_start_transpose(
                        fg, feat.ap()[r0:r0 + GROUP * P, :])
                    xtg = gin.tile([P, KC, GROUP * P], BF16, tag="xtg",
                                   name=f"xtg{g}")
                    nc.gpsimd.tensor_tensor(xtg, pg, fg, op=OP.mult)
                    state[("dma", g)] = (xtg, og)
                else:
                    pg = gin.tile([P, GROUP, D], BF16, tag="pg", name=f"pg{g}")
                    fg = gin.tile([P, GROUP, D], BF16, tag="fg", name=f"fg{g}")
                    src = pri.ap()[r0:r0 + GROUP * P, :].rearrange(
                        "(c p) d -> p c d", p=P)
                    nc.sync.dma_start(out=pg, in_=src)
                    src = feat.ap()[r0:r0 + GROUP * P, :].rearrange(
                        "(c p) d -> p c d", p=P)
                    nc.sync.dma_start(out=fg, in_=src)
                    state[("dma", g)] = (pg, fg, og)

            st = {"g": g, "h": h}
            st["mx"] = small.tile([P, SG], F32, tag="mx", name=f"mx_{n}")
            if transpose_loads:
                xtg, og = state[("dma", g)]
                zb, z_ps = compute_pair_t(xtg, h, st["mx"])
            else:
                pg, fg, og = state[("dma", g)]
                zb, z_list = compute_pair(pg, fg, h, st["mx"])
            st["og"] = og
            st["zbs"] = [zb[:, j, :] for j in range(SG)]
            st["zps"] = z_list
            state[n] = st

        def stage_b(n):
            """Newton iterations + final + output for scalar-group n."""
            st = state.pop(n)
            g, h, og = st["g"], st["h"], st["og"]
            zbs, zps = st["zbs"], st["zps"]

            taus = [small.tile([P, SG], F32, tag=f"tau{i}", name=f"tau{i}_{n}")
                    for i in range(N_BF16_ITERS + 1)]
            k_g = [small.tile([P, SG], F32, tag=f"k{i}", name=f"k{i}_{n}")
                   for i in range(N_BF16_ITERS + 1)]
            sm_g = [small.tile([P, SG], F32, tag=f"sm{i}", name=f"sm{i}_{n}")
                    for i in range(N_BF16_ITERS)]
            s4_g = small.tile([P, SG], F32, tag="s4", name=f"s4_{n}")

            # tau0 = A*mx + B
            nc.gpsimd.tensor_scalar(taus[0], st["mx"], TAU_A, TAU_B, OP.mult, OP.add)

            for i in range(N_BF16_ITERS):
                # w = -512*tau - 1 so that sum(max(zb,tau)) + w = s_relu - 1
                # with s_relu = sum(relu(zb - tau)) = S - k*tau
                w = small.tile([P, SG], F32, tag="w", name=f"w{n}_{i}")
                nc.gpsimd.tensor_scalar(w, taus[i], -512.0, -1.0, OP.mult, OP.add)
                for j in range(SG):
                    junk = junkp.tile([P, D], BF16, tag="junk", name=f"j{n}_{i}_{j}")
                    nc.vector.tensor_scalar(
                        junk, zbs[j], taus[i][:, j:j + 1], None, OP.is_gt,
                        op1=OP.add, accum_out=k_g[i][:, j:j + 1],
                    )
                    junk2 = junkp.tile([P, D], BF16, tag="junk2", name=f"i{n}_{i}_{j}")
                    # sm = sum(max(zb, tau)) + w = s_relu - 1
                    nc.vector.tensor_scalar(
                        junk2, zbs[j], taus[i][:, j:j + 1], w[:, j:j + 1], OP.max,
                        op1=OP.add, accum_out=sm_g[i][:, j:j + 1],
                    )
                # tau' = tau + (s_relu - 1)/k
                rk = small.tile([P, SG], F32, tag="rk", name=f"rk{n}_{i}")
                nc.vector.reciprocal(rk, k_g[i])
                dd = small.tile([P, SG], F32, tag="dd", name=f"dd{n}_{i}")
                nc.gpsimd.tensor_mul(dd, sm_g[i], rk)
                nc.gpsimd.tensor_add(taus[i + 1], dd, taus[i])

            # final f32 step fused with the output (double-relu):
            #   k4 = #{zb > tau3};  r1 = relu(z - tau3), s4 = sum(r1) (ACT)
            #   delta = (s4-1)/k4;  out = relu(r1 - delta)            (ACT)
            tau3 = taus[N_BF16_ITERS]
            ntau = small.tile([P, SG], F32, tag="ntau", name=f"ntau{n}")
            nc.gpsimd.tensor_scalar(ntau, tau3, -1.0, None, OP.mult)
            r1s = []
            for j in range(SG):
                junk = junkp.tile([P, D], BF16, tag="junk", name=f"jf{n}_{j}")
                nc.vector.tensor_scalar(
                    junk, zbs[j], tau3[:, j:j + 1], None, OP.is_gt,
                    op1=OP.add, accum_out=k_g[N_BF16_ITERS][:, j:j + 1],
                )
                r1 = work.tile([P, D], F32, tag="r1", name=f"r1_{n}_{j}")
                nc.scalar.activation(
                    r1, zps[j], RELU, bias=ntau[:, j:j + 1], scale=1.0,
                    accum_out=s4_g[:, j:j + 1],
                )
                r1s.append(r1)
            rk4 = small.tile([P, SG], F32, tag="rk4", name=f"rk4_{n}")
            nc.vector.reciprocal(rk4, k_g[N_BF16_ITERS])
            ndlt = small.tile([P, SG], F32, tag="ndlt", name=f"ndlt{n}")
            # ndlt = -(s4-1)/k4 = (1-s4)*rk4
            u4 = small.tile([P, SG], F32, tag="u4", name=f"u4_{n}")
            nc.gpsimd.tensor_scalar(u4, s4_g, -1.0, 1.0, OP.mult, OP.add)
            nc.gpsimd.tensor_mul(ndlt, u4, rk4)
            for j in range(SG):
                nc.scalar.activation(
                    og[:, h * SG + j, :], r1s[j], RELU,
                    bias=ndlt[:, j:j + 1], scale=1.0,
                )
            if h == GROUP // SG - 1:
                pending_stores.append((g, og))

        for _rep in range(repeats):
            state.clear()
            for n in range(NSG + 1):
                if n < NSG:
                    stage_a(n)
                if n >= 1:
                    stage_b(n - 1)
                if len(pending_stores) > 1:
                    flush_store()
            while pending_stores:
                flush_store()

    nc.finalize()
    return nc


def _build_floor():
    """Stripped variant for A/B timing: full data path but no Newton
    iterations and a single relu output at tau0. NOT numerically correct."""
    nc = bacc.Bacc("TRN2", target_bir_lowering=False, debug=False)

    pri = nc.dram_tensor("priors", [ROWS, D], BF16, kind="ExternalInput")
    feat = nc.dram_tensor("processed_feat", [ROWS, D], BF16, kind="ExternalInput")
    wt = nc.dram_tensor("w_t", [D, D], BF16, kind="ExternalInput")
    be = nc.dram_tensor("b_eff", [1, D], BF16, kind="ExternalInput")
    out = nc.dram_tensor("out", [ROWS, D], F32, kind="ExternalOutput")

    with tile.TileContext(nc) as tc, ExitStack() as ctx:
        consts = ctx.enter_context(tc.tile_pool(name="consts", bufs=1))
        gin = ctx.enter_context(tc.tile_pool(name="gin", bufs=4))
        gout = ctx.enter_context(tc.tile_pool(name="gout", bufs=4))
        work = ctx.enter_context(tc.tile_pool(name="work", bufs=10))
        junkp = ctx.enter_context(tc.tile_pool(name="junkp", bufs=16))
        small = ctx.enter_context(tc.tile_pool(name="small", bufs=10))
        if transpose_loads:
            psA = None
            psB = ctx.enter_context(tc.tile_pool(name="psB", bufs=4, space="PSUM"))
        else:
            psA = ctx.enter_context(tc.tile_pool(name="psA", bufs=2, space="PSUM"))
            psB = ctx.enter_context(tc.tile_pool(name="psB", bufs=6, space="PSUM"))

        wt_s = consts.tile([P, KC, D], BF16)
        nc.sync.dma_start(out=wt_s, in_=wt.ap().rearrange("(c p) e -> p c e", p=P))
        be_s = consts.tile([1, D], BF16)
        nc.sync.dma_start(out=be_s, in_=be.ap())
        ones = consts.tile([1, P], BF16)
        nc.vector.memset(ones, 1.0)
        ident = consts.tile([P, P], BF16)
        make_identity(nc, ident)

        for g in range(N_GROUPS):
            r0 = g * GROUP * P
            pg = gin.tile([P, GROUP, D], BF16, tag="pg", name=f"pg{g}")
            fg = gin.tile([P, GROUP, D], BF16, tag="fg", name=f"fg{g}")
            og = gout.tile([P, GROUP, D], F32, tag="og", name=f"og{g}")
            src = pri.ap()[r0:r0 + GROUP * P, :].rearrange("(c p) d -> p c d", p=P)
            nc.sync.dma_start(out=pg, in_=src)
            src = feat.ap()[r0:r0 + GROUP * P, :].rearrange("(c p) d -> p c d", p=P)
            nc.sync.dma_start(out=fg, in_=src)

            for h in range(GROUP // SG):
                z_ps = psB.tile([P, SG, D], F32, tag="z_ps", name=f"z{g}_{h}")
                mx_g = small.tile([P, SG], F32, tag="mx", name=f"mx{g}_{h}")
                zbs = []
                for j in range(SG):
                    c = h * SG + j
                    x = work.tile([P, D], BF16, tag="x", name=f"x{g}_{c}")
                    nc.gpsimd.tensor_tensor(x, pg[:, c, :], fg[:, c, :], op=OP.mult)
                    xt_ps = psA.tile([P, D], BF16, tag="xt_ps", name=f"xp{g}_{c}")
                    for cc in range(KC):
                        nc.tensor.transpose(
                            xt_ps[:, cc * P:(cc + 1) * P],
                            x[:, cc * P:(cc + 1) * P], ident)
                    xt = work.tile([P, D], BF16, tag="xt", name=f"xt{g}_{c}")
                    nc.vector.tensor_copy(xt, xt_ps)
                    for cc in range(KC):
                        nc.tensor.matmul(
                            z_ps[:, j, :], xt[:, cc * P:(cc + 1) * P],
                            wt_s[:, cc, :], start=(cc == 0), stop=False)
                    nc.tensor.matmul(z_ps[:, j, :], ones, be_s,
                                     start=False, stop=True)
                zb = work.tile([P, SG, D], BF16, tag="zb", name=f"zb{g}_{h}")
                nc.scalar.copy(zb, z_ps)
                for j in range(SG):
                    junk = junkp.tile([P, D], BF16, tag="junk", name=f"jm{g}_{h}_{j}")
                    nc.vector.tensor_scalar(
                        junk, zb[:, j, :], -1e30, None, OP.max,
                        op1=OP.max, accum_out=mx_g[:, j:j + 1])
                ntau = small.tile([P, SG], F32, tag="ntau", name=f"nt{g}_{h}")
                nc.gpsimd.tensor_scalar(ntau, mx_g, -TAU_A, -TAU_B, OP.mult, OP.add)
                for j in range(SG):
                    nc.scalar.activation(
                        og[:, h * SG + j, :], z_ps[:, j, :], RELU,
                        bias=ntau[:, j:j + 1], scale=1.0)

            dst = out.ap()[r0:r0 + GROUP * P, :].rearrange("(c p) d -> p c d", p=P)
            nc.sync.dma_start(out=dst, in_=og)

    nc.finalize()
    return nc


def _run_spmd(nc, in_maps, n_cores, reps=0):
    """Execute the Bass graph SPMD on `n_cores` axon-attached NeuronCores.

    Replicates bass2jax.run_bass_via_pjrt but without output-buffer donation,
    so the jitted executable can be invoked repeatedly on device-resident
    inputs for wall-clock timing (reps > 0 stores best-of-reps seconds in
    LAST_WALL_S).
    """
    global LAST_WALL_S
    import time

    import jax
    from jax.sharding import Mesh, NamedSharding, PartitionSpec
    from jax.experimental.shard_map import shard_map

    from concourse import bass2jax
    from concourse.bass2jax import _bass_exec_p, install_neuronx_cc_hook

    install_neuronx_cc_hook()

    partition_name = nc.partition_id_tensor.name if nc.partition_id_tensor else None

    in_names, out_names, out_avals, zero_outs = [], [], [], []
    for alloc in nc.m.functions[0].allocations:
        if not isinstance(alloc, mybir.MemoryLocationSet):
            continue
        name = alloc.memorylocations[0].name
        if alloc.kind == "ExternalInput":
            if name != partition_name:
                in_names.append(name)
        elif alloc.kind == "ExternalOutput":
            shape = tuple(alloc.tensor_shape)
            dtype = mybir.dt.np(alloc.dtype)
            out_names.append(name)
            out_avals.append(jax.core.ShapedArray(shape, dtype))
            zero_outs.append(np.zeros(shape, dtype))
    n_params = len(in_names)
    all_names = in_names + out_names
    if partition_name is not None:
        all_names = all_names + [partition_name]

    def _exec_once(args):
        operands = list(args)
        if partition_name is not None:
            operands.append(bass2jax.partition_id_tensor())
        return _bass_exec_p.bind(
            *operands,
            out_avals=tuple(out_avals),
            in_names=tuple(all_names),
            out_names=tuple(out_names),
            lowering_input_output_aliases=(),
            sim_require_finite=True,
            sim_require_nnan=True,
            nc=nc,
        )

    def _body(*args):
        return tuple(_exec_once(args))

    def _make_chained(k, be_idx):
        # each call gets a distinct b_eff parameter so XLA cannot CSE the
        # otherwise-identical custom calls; timing is data-independent.
        def _body_k(*args):
            base = list(args[:-k])
            bes = args[-k:]
            allouts = []
            for i in range(k):
                ops = list(base)
                ops[be_idx] = bes[i]
                allouts.extend(_exec_once(ops))
            return tuple(allouts)
        return _body_k

    devices = jax.devices()[:n_cores]
    mesh = Mesh(np.asarray(devices), ("core",))
    spec = PartitionSpec("core")
    n_args = n_params + len(out_names)
    fn = jax.jit(
        shard_map(
            _body,
            mesh=mesh,
            in_specs=(spec,) * n_args,
            out_specs=(spec,) * len(out_names),
            check_rep=False,
        ),
        keep_unused=True,
    )
    sharding = NamedSharding(mesh, spec)
    concat_in = [
        jax.device_put(
            np.concatenate([np.asarray(in_maps[c][k]) for c in range(n_cores)], 0),
            sharding,
        )
        for k in in_names
    ]
    concat_zeros = [
        jax.device_put(np.zeros((n_cores * z.shape[0], *z.shape[1:]), z.dtype), sharding)
        for z in zero_outs
    ]
    args = concat_in + concat_zeros
    outs = fn(*args)  # first call compiles
    jax.block_until_ready(outs)

    if reps > 0:
      try:
        CH = int(os.environ.get("BASS_KERNEL_CHAIN", "16"))
        be_idx = in_names.index("b_eff")
        fn_k = jax.jit(
            shard_map(
                _make_chained(CH, be_idx),
                mesh=mesh,
                in_specs=(spec,) * (n_args + CH),
                out_specs=(spec,) * (len(out_names) * CH),
                check_rep=False,
            ),
            keep_unused=True,
        )
        be_np = np.concatenate(
            [np.asarray(in_maps[c]["b_eff"]) for c in range(n_cores)], 0)
        bes = [jax.device_put(be_np.copy(), sharding) for _ in range(CH)]
        args_k = args + bes
        o2 = fn_k(*args_k)
        jax.block_until_ready(o2)

        def best(f, a, n):
            ts = []
            for _ in range(n):
                t0 = time.perf_counter()
                jax.block_until_ready(f(*a))
                ts.append(time.perf_counter() - t0)
            return min(ts)

        t1 = best(fn, args, reps)
        tk = best(fn_k, args_k, reps)
        LAST_WALL_S = (tk - t1) / (CH - 1)
        print(f"[timing] t1={t1*1e3:.2f}ms t{CH}={tk*1e3:.2f}ms "
              f"-> per-exec {LAST_WALL_S*1e6:.0f}us")
      except Exception as e:
        print(f"[timing] skipped: {str(e)[:120]}")

    return [
        {
            k: np.asarray(outs[i]).reshape(n_cores, *out_avals[i].shape)[c]
            for i, k in enumerate(out_names)
        }
        for c in range(n_cores)
    ]


def kernel(priors, processed_feat, bn_gamma, bn_beta, bn_mean, bn_var, fc_w, fc_b):
    global LAST_RESULTS
    import ml_dtypes

    BF = ml_dtypes.bfloat16
    priors = np.ascontiguousarray(np.asarray(priors, dtype=np.float32).astype(BF))
    processed_feat = np.ascontiguousarray(
        np.asarray(processed_feat, dtype=np.float32).astype(BF)
    )

    # Fold BatchNorm (eval) into the Linear layer, in float64 for accuracy.
    g64 = np.asarray(bn_gamma, np.float64)
    b64 = np.asarray(bn_beta, np.float64)
    m64 = np.asarray(bn_mean, np.float64)
    v64 = np.asarray(bn_var, np.float64)
    w64 = np.asarray(fc_w, np.float64)
    fb64 = np.asarray(fc_b, np.float64)
    scale = g64 / np.sqrt(v64 + BN_EPS)
    shift = b64 - m64 * scale
    w_eff = w64 * scale[None, :]
    b_eff = fb64 + w64 @ shift
    w_t = np.ascontiguousarray(w_eff.T.astype(np.float32).astype(BF))
    b_eff = np.ascontiguousarray(b_eff.astype(np.float32).astype(BF)[None, :])

    nc = _build_bass()

    in_maps = []
    for i in range(N_CORES):
        in_maps.append({
            "priors": priors[i * ROWS:(i + 1) * ROWS],
            "processed_feat": processed_feat[i * ROWS:(i + 1) * ROWS],
            "w_t": w_t,
            "b_eff": b_eff,
        })

    reps = int(os.environ.get("BASS_KERNEL_REPS", "0"))
    results = _run_spmd(nc, in_maps, N_CORES, reps=reps)
    LAST_RESULTS = results

    out = np.concatenate([results[i]["out"] for i in range(N_CORES)], axis=0)
    return out
